# revision 1
# baseline (speedup 1.0000x reference)
"""DLRM embedding-lookup + tiny MLP kernel for 8 TRN2 NeuronCores.

Strategy: data-parallel over the batch (2048 rows/core); the embedding
tables are concatenated into one big table and replicated to every core's
HBM, so each core gathers its own 2048*28 embedding rows locally with no
collectives.  The gathered rows land batch-on-partitions; PE-mode
transposes flip each 128x128 block so the feature dim sits on partitions,
then the 1792->128 matmul accumulates in PSUM, relu, a 128->1 matmul and
sigmoid finish the row scores.

Tables/weights are stored bf16 on device (host converts); accumulation is
f32 in PSUM.
"""

import numpy as np
import ml_dtypes

import concourse.bass as bass
import concourse.bacc as bacc
import concourse.mybir as mybir
import concourse.tile as tile
from concourse.bass_utils import run_bass_kernel_spmd
from concourse.masks import make_identity

# problem shape (hardcoded per contract)
B = 16384
N_CORES = 8
BC = B // N_CORES          # 2048 batch rows per core
EMB = 64
NS = 26                    # sparse features
NF = NS + 2                # total embeddings per row (user, item, sparse)
USER_V = 1_000_000
ITEM_V = 1_000_000
SPARSE_V = 100_000
VT = USER_V + ITEM_V + NS * SPARSE_V   # 4.6M rows in the concat table
MLP_IN = EMB * NF          # 1792
HIDDEN = 128
KBLK = MLP_IN // 128       # 14 contraction blocks
ST = 512                   # supertile batch rows (4 subtiles of 128)
NSUB = ST // 128           # 4
NST = BC // ST             # 4 supertiles per core

TABLE_DT = mybir.dt.bfloat16
TABLE_NP = ml_dtypes.bfloat16
F32 = mybir.dt.float32
I32 = mybir.dt.int32

_PROG = None


def _build() -> bass.Bass:
    nc = bacc.Bacc()

    table = nc.declare_dram_parameter("table", [VT, EMB], TABLE_DT, isOutput=False)
    w1 = nc.declare_dram_parameter("w1", [128, KBLK * 128], TABLE_DT, isOutput=False)
    w2 = nc.declare_dram_parameter("w2", [128, 1], TABLE_DT, isOutput=False)
    b1 = nc.declare_dram_parameter("b1", [128, 1], F32, isOutput=False)
    b2 = nc.declare_dram_parameter("b2", [1, 1], F32, isOutput=False)
    idx = nc.declare_dram_parameter("idx", [128, NST * NSUB * NF], I32, isOutput=False)
    out = nc.declare_dram_parameter("out", [NST, ST], F32, isOutput=True)

    with tile.TileContext(nc) as tc:
        with (
            tc.tile_pool(name="const", bufs=1) as cpool,
            tc.tile_pool(name="x", bufs=2) as xpool,
            tc.tile_pool(name="xt", bufs=2) as xtpool,
            tc.tile_pool(name="h", bufs=2) as hpool,
            tc.tile_pool(name="y", bufs=2) as ypool,
            tc.tile_pool(name="pt", bufs=4, space="PSUM") as ptpool,
            tc.tile_pool(name="ph", bufs=2, space="PSUM") as phpool,
            tc.tile_pool(name="po", bufs=2, space="PSUM") as popool,
        ):
            w1_sb = cpool.tile([128, KBLK * 128], TABLE_DT)
            nc.sync.dma_start(out=w1_sb[:], in_=w1[:])
            w2_sb = cpool.tile([128, 1], TABLE_DT)
            nc.sync.dma_start(out=w2_sb[:], in_=w2[:])
            b1_sb = cpool.tile([128, 1], F32)
            nc.sync.dma_start(out=b1_sb[:], in_=b1[:])
            b2_sb = cpool.tile([1, 1], F32)
            nc.sync.dma_start(out=b2_sb[:], in_=b2[:])
            idx_sb = cpool.tile([128, NST * NSUB * NF], I32)
            nc.sync.dma_start(out=idx_sb[:], in_=idx[:])
            ident = cpool.tile([128, 128], TABLE_DT)
            make_identity(nc, ident[:])

            copy_flip = 0
            for t in range(NST):
                x_sb = xpool.tile([128, NSUB * NF * EMB], TABLE_DT)
                # HW consumes ONE offset per partition row per indirect DMA
                # (sim generalizes to many, silicon does not) — so issue one
                # gather of 128 rows per (subtile, feature) slot column.
                for j in range(NSUB * NF):
                    nc.gpsimd.indirect_dma_start(
                        out=x_sb[:, j * EMB:(j + 1) * EMB],
                        out_offset=None,
                        in_=table[:],
                        in_offset=bass.IndirectOffsetOnAxis(
                            ap=idx_sb[:, t * NSUB * NF + j:
                                      t * NSUB * NF + j + 1],
                            axis=0,
                        ),
                    )
                # xt[p_f, c*ST + s*128 + b] = X[b_row(s, b), c*128 + p_f]
                xt_sb = xtpool.tile([128, KBLK * ST], TABLE_DT)
                xt_3d = xt_sb[:].rearrange("p (c b) -> p c b", b=ST)
                for s in range(NSUB):
                    for cg in range((KBLK + 3) // 4):  # groups of <=4 blocks
                        c0 = cg * 4
                        nb = min(4, KBLK - c0)
                        pt = ptpool.tile([128, 512], TABLE_DT, tag="pt")
                        for j in range(nb):
                            c = c0 + j
                            nc.tensor.transpose(
                                out=pt[:, j * 128:(j + 1) * 128],
                                in_=x_sb[:, s * MLP_IN + c * 128:
                                         s * MLP_IN + (c + 1) * 128],
                                identity=ident[:],
                            )
                        dst = xt_3d[:, c0:c0 + nb, s * 128:(s + 1) * 128]
                        src = pt[:, :nb * 128].rearrange(
                            "p (c b) -> p c b", b=128)
                        if copy_flip % 2 == 0:
                            nc.vector.tensor_copy(out=dst, in_=src)
                        else:
                            nc.scalar.activation(
                                out=dst, in_=src,
                                func=mybir.ActivationFunctionType.Copy)
                        copy_flip += 1

                ph = phpool.tile([128, ST], F32)
                for c in range(KBLK):
                    nc.tensor.matmul(
                        out=ph[:],
                        lhsT=w1_sb[:, c * 128:(c + 1) * 128],
                        rhs=xt_sb[:, c * ST:(c + 1) * ST],
                        start=(c == 0),
                        stop=(c == KBLK - 1),
                    )
                h_sb = hpool.tile([128, ST], TABLE_DT)
                nc.scalar.activation(
                    out=h_sb[:], in_=ph[:],
                    func=mybir.ActivationFunctionType.Relu,
                    bias=b1_sb[:, 0:1],
                )
                po = popool.tile([1, ST], F32)
                nc.tensor.matmul(
                    out=po[:], lhsT=w2_sb[:], rhs=h_sb[:],
                    start=True, stop=True,
                )
                y_sb = ypool.tile([1, ST], F32)
                nc.scalar.activation(
                    out=y_sb[:], in_=po[:],
                    func=mybir.ActivationFunctionType.Sigmoid,
                    bias=b2_sb[0:1, 0:1],
                )
                nc.sync.dma_start(out=out[t:t + 1, :], in_=y_sb[:])

    nc.compile()
    return nc


def _get_prog() -> bass.Bass:
    global _PROG
    if _PROG is None:
        _PROG = _build()
    return _PROG


def make_in_maps(user_ids, item_ids, sparse_features, user_emb, item_emb,
                 sparse_tables, W1, b1, W2, b2):
    user_ids = np.asarray(user_ids)
    item_ids = np.asarray(item_ids)
    sparse_features = np.asarray(sparse_features)

    big_table = np.concatenate(
        [np.asarray(user_emb, dtype=np.float32),
         np.asarray(item_emb, dtype=np.float32),
         np.asarray(sparse_tables, dtype=np.float32).reshape(-1, EMB)],
        axis=0,
    ).astype(TABLE_NP)

    w1_host = (np.asarray(W1, dtype=np.float32)
               .reshape(KBLK, 128, HIDDEN)
               .transpose(1, 0, 2)
               .reshape(128, KBLK * HIDDEN)
               .astype(TABLE_NP))
    w2_host = np.asarray(W2, dtype=np.float32).reshape(128, 1).astype(TABLE_NP)
    b1_host = np.asarray(b1, dtype=np.float32).reshape(128, 1)
    b2_host = np.asarray(b2, dtype=np.float32).reshape(1, 1)

    gidx = np.empty((B, NF), dtype=np.int64)
    gidx[:, 0] = user_ids
    gidx[:, 1] = USER_V + item_ids
    base = USER_V + ITEM_V
    for f in range(NS):
        gidx[:, 2 + f] = base + f * SPARSE_V + sparse_features[:, f]
    gidx = gidx.astype(np.int32)

    in_maps = []
    for c in range(N_CORES):
        rows = gidx[c * BC:(c + 1) * BC]                    # [2048, 28]
        idx_core = (rows.reshape(NST, NSUB, 128, NF)
                    .transpose(2, 0, 1, 3)
                    .reshape(128, NST * NSUB * NF)
                    .copy())
        in_maps.append({
            "table": big_table,
            "w1": w1_host,
            "w2": w2_host,
            "b1": b1_host,
            "b2": b2_host,
            "idx": idx_core,
        })
    return in_maps


def assemble_output(results) -> np.ndarray:
    parts = [np.asarray(results[c]["out"], dtype=np.float32).reshape(BC)
             for c in range(N_CORES)]
    return np.concatenate(parts).reshape(B, 1)


def kernel(**inputs) -> np.ndarray:
    nc = _get_prog()
    in_maps = make_in_maps(**inputs)
    res = run_bass_kernel_spmd(nc, in_maps, core_ids=list(range(N_CORES)))
    return assemble_output(res.results)

